# revision 37
# baseline (speedup 1.0000x reference)
"""Trainium2 Bass kernel for the BDH dense transformer (B=2, T=512, D=256, NH=4,
N=8192, 4 weight-tied layers, vocab 256).

Sharding: one (batch, head) pair per NeuronCore (2 x 4 = 8 cores). Per layer,
each core computes its head's yMLP partial (T, D); the 4 cores of a batch group
exchange partials with an AllGather and reduce locally, then every core in the
group redundantly applies the residual + layernorm so the activations stay
replicated within the group.

RoPE trick: the score contraction over the neuron dim N is invariant under any
permutation of N applied consistently to (encoder cols, encoder_v cols, decoder
rows, rope freqs). We de-interleave N (even indices first, then odd) so the
interleaved-pair rope becomes rotate-half form: the partner of partition-chunk
j is chunk j+/-32 — a whole-chunk offset instead of an adjacent-partition
shuffle. cos/sin tables are identical for both halves, so only (N/2, T) tables
are streamed.

All matmuls run in bf16 with fp32 PSUM accumulation; layernorms and the
residual stream are fp32.
"""

import math

import numpy as np
import ml_dtypes

import concourse.bass as bass
import concourse.mybir as mybir
import concourse.tile as tile
from concourse import bacc
from concourse import bass_utils
from concourse.masks import make_identity

BF16 = ml_dtypes.bfloat16
F32 = mybir.dt.float32
BF = mybir.dt.bfloat16

# model dims (hardcoded per the problem spec)
B, T, D, NH, VOCAB = 2, 512, 256, 4, 256
N_LAYER = 4
MLP_MULT = 128
N = D * MLP_MULT // NH          # 8192 neurons per head
LN_EPS = 1e-5
TWO_PI = 2.0 * math.pi

NCH = N // 128                   # 64 partition-chunks of the neuron dim
HCH = NCH // 2                   # 32 chunks per rotate-half half
NGRP = 16                        # rope groups of 2 lo-chunks + 2 hi-chunks
TC = T // 128                    # 4 t-chunks
DC = D // 128                    # 2 d-chunks
VC = VOCAB // 128                # 2 vocab-chunks

RG = [[0, 1, 2, 3], [4, 5, 6, 7]]

AF = mybir.ActivationFunctionType
ALU = mybir.AluOpType


def _layer_norm(nc, tmp, eps_tile, src, out_bf, out_f32=None):
    """LN over the free dim (256 wide) of a (128, 256) f32 tile (SBUF or PSUM)."""
    stats = tmp.tile([128, 6], F32, tag="bnst", bufs=2, name="ln_stats")
    nc.vector.bn_stats(stats, src)
    mv = tmp.tile([128, 2], F32, tag="bnmv", bufs=2, name="ln_mv")
    nc.vector.bn_aggr(mv, stats)
    std = tmp.tile([128, 1], F32, tag="std", bufs=2, name="ln_std")
    nc.scalar.activation(std, mv[:, 1:2], AF.Sqrt, bias=eps_tile)
    rstd = tmp.tile([128, 1], F32, tag="rstd", bufs=2, name="ln_rstd")
    nc.vector.reciprocal(rstd, std)
    first_out = out_f32 if out_f32 is not None else out_bf
    nc.vector.tensor_scalar(
        out=first_out, in0=src, scalar1=mv[:, 0:1], scalar2=rstd,
        op0=ALU.subtract, op1=ALU.mult,
    )
    if out_f32 is not None:
        nc.scalar.copy(out_bf, out_f32)


def _build_nc():
    nc = bacc.Bacc("TRN2", target_bir_lowering=False, debug=False, num_devices=8)

    # per-core external inputs
    enc_w = nc.dram_tensor("enc_w", [DC, 128, N], BF, kind="ExternalInput")
    encv_w = nc.dram_tensor("encv_w", [DC, 128, N], BF, kind="ExternalInput")
    dec_w = nc.dram_tensor("dec_w", [NCH, 128, D], BF, kind="ExternalInput")
    cos_w = nc.dram_tensor("cos_w", [HCH, 128, T], BF, kind="ExternalInput")
    sin_w = nc.dram_tensor("sin_w", [HCH, 128, T], BF, kind="ExternalInput")
    oneh_w = nc.dram_tensor("oneh_w", [VC, 128, T], BF, kind="ExternalInput")
    emb_w = nc.dram_tensor("emb_w", [VC, 128, D], BF, kind="ExternalInput")
    lmh_w = nc.dram_tensor("lmh_w", [DC, 128, VOCAB], BF, kind="ExternalInput")
    logits_o = nc.dram_tensor("logits_o", [TC, 128, VOCAB], F32, kind="ExternalOutput")

    with tile.TileContext(nc) as tc:
        with (
            tc.tile_pool(name="wpool", bufs=1) as wp,
            tc.tile_pool(name="xspool", bufs=1) as xsp,
            tc.tile_pool(name="stream", bufs=3) as stream,
            tc.tile_pool(name="rope", bufs=2) as rope,
            tc.tile_pool(name="work", bufs=1) as work,
            tc.tile_pool(name="tmp", bufs=2) as tmp,
            tc.tile_pool(name="psA", bufs=2, space="PSUM") as psA,
            tc.tile_pool(name="psS", bufs=1, space="PSUM") as psS,
            tc.tile_pool(name="psT", bufs=2, space="PSUM") as psT,
            tc.tile_pool(name="dram", bufs=1, space="DRAM") as dram,
        ):
            # ---- resident weights (small tensors first so the embedding can
            # start while the big encoder DMAs stream in) ----
            emb_sb = wp.tile([128, VC, D], BF, name="emb_sb")
            nc.sync.dma_start(emb_sb, emb_w.ap().rearrange("c p d -> p c d"))
            oneh_sb = wp.tile([128, VC, T], BF, name="oneh_sb")
            nc.sync.dma_start(oneh_sb, oneh_w.ap().rearrange("c p t -> p c t"))

            # warm up the collectives path (same payload size as the per-layer
            # AllReduce) so layer 0 doesn't pay the first-call cost; no data
            # deps, overlaps the weight DMAs
            wcc_in = dram.tile([128, TC, D], BF, tag="wccin", name="wcc_in")
            wcc_out = dram.tile([128, TC, D], BF, tag="wccout", name="wcc_out")
            wcc_sb = tmp.tile([128, TC, D], BF, tag="wcc", bufs=1, name="wcc_sb")
            nc.vector.memset(wcc_sb, 0.0)
            nc.sync.dma_start(wcc_in, wcc_sb)

            nc.gpsimd.collective_compute(
                "AllReduce", ALU.add, replica_groups=RG,
                ins=[wcc_in.opt()], outs=[wcc_out.opt()],
            )
            lmh_sb = wp.tile([128, DC, VOCAB], BF, name="lmh_sb")
            nc.sync.dma_start(lmh_sb, lmh_w.ap().rearrange("c p v -> p c v"))
            ident = wp.tile([128, 128], BF, name="ident")
            make_identity(nc, ident)
            eps_tile = wp.tile([128, 1], F32, name="eps_tile")
            nc.vector.memset(eps_tile, LN_EPS)
            # encoder DMAs split into column slices so phase A's first matmuls
            # only wait for the slice they read (subtile deps)
            enc_sb = []
            encv_sb = []
            for dc in range(DC):
                e = wp.tile([128, N], BF, tag=f"enc{dc}", name=f"enc_sb{dc}")
                enc_sb.append(e)
                ev = wp.tile([128, N], BF, tag=f"encv{dc}", name=f"encv_sb{dc}")
                encv_sb.append(ev)
            NSL = 8
            sl = N // NSL
            for s in range(NSL):
                for dc in range(DC):
                    nc.sync.dma_start(
                        enc_sb[dc][:, s * sl:(s + 1) * sl],
                        enc_w.ap()[dc, :, s * sl:(s + 1) * sl],
                    )
            for s in range(NSL):
                for dc in range(DC):
                    nc.sync.dma_start(
                        encv_sb[dc][:, s * sl:(s + 1) * sl],
                        encv_w.ap()[dc, :, s * sl:(s + 1) * sl],
                    )

            # ---- persistent activations ----
            xs = xsp.tile([128, NCH, T], BF, name="xs")          # x_sparse / xy gate
            smask = work.tile([128, TC, T], BF, name="smask")    # masked scores (lhsT)
            x_f32 = work.tile([128, TC, D], F32, name="x_f32")
            x_bf = work.tile([128, TC, D], BF, name="x_bf")
            xT = work.tile([128, DC, T], BF, name="xT")
            ykv_bf = work.tile([128, TC, D], BF, name="ykv_bf")
            ykvT = work.tile([128, DC, T], BF, name="ykvT")
            part_bf = work.tile([128, TC, D], BF, name="part_bf")

            # zero the always-zero lower-left region of the masked score tiles
            for i in range(1, TC):
                nc.vector.memset(smask[:, i, 0:128 * i], 0.0)

            def transpose_to(dst, src_bf):
                # src_bf: (128, TC, D) bf16 (t-part, d-free) -> dst (128, DC, T)
                for j in range(TC):
                    for dc in range(DC):
                        tp = psT.tile([128, 128], BF, tag="tr", name="tp_ps")
                        nc.tensor.transpose(
                            tp, src_bf[:, j, 128 * dc:128 * (dc + 1)], ident
                        )
                        nc.scalar.copy(dst[:, dc, 128 * j:128 * (j + 1)], tp)

            # ---- embedding: x0 = LN(onehot.T @ embed) ----
            for j in range(TC):
                x0 = psT.tile([128, D], F32, tag="tr", name="x0_ps")
                for vc in range(VC):
                    nc.tensor.matmul(
                        x0, lhsT=oneh_sb[:, vc, 128 * j:128 * (j + 1)],
                        rhs=emb_sb[:, vc, :],
                        start=(vc == 0), stop=(vc == VC - 1),
                    )
                _layer_norm(nc, tmp, eps_tile, x0, x_bf[:, j, :], x_f32[:, j, :])
            transpose_to(xT, x_bf)

            # ---- layers ----
            # phase-A chunk order interleaves the rotate-half partners
            # (0,32,1,33,...) so rope group g unblocks after 4g+4 chunks
            a_order = [k for pair in zip(range(HCH), range(HCH, NCH)) for k in pair]

            for layer in range(N_LAYER):
                # phase A: x_latent^T = enc^T @ x^T ; x_sparse = relu
                with nc.named_scope(f"L{layer}_A"):
                    for ki, k in enumerate(a_order):
                        lat = psA.tile([128, T], F32, tag="lat", name="lat_ps")
                        if ki < 8:
                            # split over T so the first half only depends on
                            # the first two t-chunks of xT (overlaps the E
                            # tail of the previous layer); costs extra
                            # LDWEIGHTS so only the leading chunks do it
                            for h in range(2):
                                hs = slice(256 * h, 256 * (h + 1))
                                for dc in range(DC):
                                    nc.tensor.matmul(
                                        lat[:, hs],
                                        lhsT=enc_sb[dc][:, 128 * k:128 * (k + 1)],
                                        rhs=xT[:, dc, hs],
                                        start=(dc == 0), stop=(dc == DC - 1),
                                    )
                        else:
                            for dc in range(DC):
                                nc.tensor.matmul(
                                    lat,
                                    lhsT=enc_sb[dc][:, 128 * k:128 * (k + 1)],
                                    rhs=xT[:, dc, :],
                                    start=(dc == 0), stop=(dc == DC - 1),
                                )
                        nc.scalar.activation(xs[:, k, :], lat, AF.Relu)

                # phase B: rope + scores S = QR^T QR (upper-triangular blocks only)
                spsum = [
                    psS.tile([128, T], F32, tag=f"s{i}", name=f"score_ps{i}")
                    for i in range(TC)
                ]
                GW = 4           # rope group width (chunks per half)
                NG = HCH // GW
                with nc.named_scope(f"L{layer}_B"):
                    for g in range(NG):
                        lo = slice(GW * g, GW * (g + 1))
                        hi = slice(GW * g + HCH, GW * (g + 1) + HCH)
                        cosg = rope.tile([128, GW, T], BF, tag="cosg", name="cosg")
                        nc.sync.dma_start(
                            cosg,
                            cos_w.ap()[GW * g:GW * (g + 1)].rearrange("c p t -> p c t"),
                        )
                        sing = rope.tile([128, GW, T], BF, tag="sing", name="sing")
                        nc.sync.dma_start(
                            sing,
                            sin_w.ap()[GW * g:GW * (g + 1)].rearrange("c p t -> p c t"),
                        )
                        qrlo = rope.tile([128, GW, T], BF, tag="qrlo", name="qrlo")
                        qrhi = rope.tile([128, GW, T], BF, tag="qrhi", name="qrhi")
                        ta = rope.tile([128, GW, T], BF, tag="ta", bufs=1, name="ropeta")
                        tb = rope.tile([128, GW, T], BF, tag="tb", bufs=1, name="ropetb")
                        nc.vector.tensor_mul(ta, xs[:, lo, :], cosg)
                        nc.vector.tensor_mul(tb, xs[:, hi, :], sing)
                        nc.vector.tensor_sub(qrlo, ta, tb)
                        ta2 = rope.tile([128, GW, T], BF, tag="ta", bufs=1, name="ropeta2")
                        tb2 = rope.tile([128, GW, T], BF, tag="tb", bufs=1, name="ropetb2")
                        nc.vector.tensor_mul(ta2, xs[:, hi, :], cosg)
                        nc.vector.tensor_mul(tb2, xs[:, lo, :], sing)
                        nc.vector.tensor_add(qrhi, ta2, tb2)
                        for qr, base in ((qrlo, GW * g), (qrhi, GW * g + HCH)):
                            for kk in range(GW):
                                first = (g == 0) and (qr is qrlo) and (kk == 0)
                                last = (g == NG - 1) and (qr is qrhi) and (kk == GW - 1)
                                for i in range(TC):
                                    nc.tensor.matmul(
                                        spsum[i][:, 128 * i:T],
                                        lhsT=qr[:, kk, 128 * i:128 * (i + 1)],
                                        rhs=qr[:, kk, 128 * i:T],
                                        start=first, stop=last,
                                    )

                # phase C: mask scores, attention out, LN, transpose
                with nc.named_scope(f"L{layer}_C"):
                    for i in range(TC):
                        if i % 2 == 0:
                            nc.vector.tensor_copy(
                                out=smask[:, i, 128 * i:T],
                                in_=spsum[i][:, 128 * i:T],
                            )
                        else:
                            nc.scalar.copy(
                                smask[:, i, 128 * i:T], spsum[i][:, 128 * i:T]
                            )
                        diag = smask[:, i, 128 * i:128 * (i + 1)]
                        nc.gpsimd.affine_select(
                            out=diag, in_=diag, pattern=[[1, 128]], base=0,
                            channel_multiplier=-1, compare_op=ALU.is_gt, fill=0.0,
                        )
                    for j in range(TC):
                        att = psT.tile([128, D], F32, tag="tr", name="att_ps")
                        for i in range(j + 1):
                            nc.tensor.matmul(
                                att, lhsT=smask[:, i, 128 * j:128 * (j + 1)],
                                rhs=x_bf[:, i, :],
                                start=(i == 0), stop=(i == j),
                            )
                        _layer_norm(nc, tmp, eps_tile, att, ykv_bf[:, j, :])
                    transpose_to(ykvT, ykv_bf)

                # phase D: y_latent, relu, gate, yMLP partial
                mlp = [
                    psS.tile([128, D], F32, tag=f"s{j}", name=f"mlp_ps{j}")
                    for j in range(TC)
                ]
                with nc.named_scope(f"L{layer}_D"):
                    dec_pair = None
                    for k in range(NCH):
                        if k % 2 == 0:
                            dec_pair = stream.tile(
                                [128, 2, D], BF, tag="dec", bufs=4, name="dec_t"
                            )
                            nc.sync.dma_start(
                                dec_pair,
                                dec_w.ap()[k:k + 2].rearrange("c p d -> p c d"),
                            )
                        dec_t = dec_pair[:, k % 2, :]
                        ylat = psA.tile([128, T], F32, tag="lat", name="ylat_ps")
                        if k < 4:
                            # T-halves so the first ylats only need the first
                            # two transposed yKV chunks (overlaps phase C)
                            for h in range(2):
                                hs = slice(256 * h, 256 * (h + 1))
                                for dc in range(DC):
                                    nc.tensor.matmul(
                                        ylat[:, hs],
                                        lhsT=encv_sb[dc][:, 128 * k:128 * (k + 1)],
                                        rhs=ykvT[:, dc, hs],
                                        start=(dc == 0), stop=(dc == DC - 1),
                                    )
                        else:
                            for dc in range(DC):
                                nc.tensor.matmul(
                                    ylat,
                                    lhsT=encv_sb[dc][:, 128 * k:128 * (k + 1)],
                                    rhs=ykvT[:, dc, :],
                                    start=(dc == 0), stop=(dc == DC - 1),
                                )
                        ys = stream.tile([128, T], BF, tag="ys", name="ys_t")
                        nc.scalar.activation(ys, ylat, AF.Relu)
                        nc.vector.tensor_mul(xs[:, k, :], xs[:, k, :], ys)
                        for j in range(TC):
                            nc.tensor.matmul(
                                mlp[j], lhsT=xs[:, k, 128 * j:128 * (j + 1)],
                                rhs=dec_t,
                                start=(k == 0), stop=(k == NCH - 1),
                            )

                # phase E: exchange partials, reduce, residual + LN, transpose
                with nc.named_scope(f"L{layer}_E"):
                    for j in range(TC):
                        if j % 2 == 0:
                            nc.scalar.copy(part_bf[:, j, :], mlp[j])
                        else:
                            nc.vector.tensor_copy(out=part_bf[:, j, :], in_=mlp[j])
                    cc_in = dram.tile([128, TC, D], BF, tag="ccin", name="cc_in")
                    cc_out = dram.tile([128, TC, D], BF, tag="ccout", name="cc_out")
                    nc.sync.dma_start(cc_in, part_bf)
                    nc.gpsimd.collective_compute(
                        "AllReduce", ALU.add, replica_groups=RG,
                        ins=[cc_in.opt()], outs=[cc_out.opt()],
                    )
                    ag = tmp.tile([128, TC, D], BF, tag="ag", bufs=1, name="ag")
                    nc.sync.dma_start(ag[:, 0:2, :], cc_out[:, 0:2, :])
                    nc.sync.dma_start(ag[:, 2:4, :], cc_out[:, 2:4, :])
                    for j in range(TC):
                        xsum = tmp.tile([128, D], F32, tag="xsum", name="xsum")
                        nc.vector.tensor_add(xsum, ag[:, j, :], x_f32[:, j, :])
                        _layer_norm(
                            nc, tmp, eps_tile, xsum,
                            x_bf[:, j, :], x_f32[:, j, :],
                        )
                        for dc in range(DC):
                            tp = psT.tile([128, 128], BF, tag="tr", name="tp_ps")
                            nc.tensor.transpose(
                                tp, x_bf[:, j, 128 * dc:128 * (dc + 1)], ident
                            )
                            nc.scalar.copy(
                                xT[:, dc, 128 * j:128 * (j + 1)], tp
                            )
                        if layer == N_LAYER - 1:
                            # lm head per chunk, overlapping the rest of the
                            # final exchange tail
                            lg = psT.tile([128, VOCAB], F32, tag="tr", name="lg_ps")
                            for dc in range(DC):
                                nc.tensor.matmul(
                                    lg, lhsT=xT[:, dc, 128 * j:128 * (j + 1)],
                                    rhs=lmh_sb[:, dc, :],
                                    start=(dc == 0), stop=(dc == DC - 1),
                                )
                            lgs = tmp.tile([128, VOCAB], F32, tag="lgs", name="lg_sb")
                            nc.scalar.copy(lgs, lg)
                            nc.sync.dma_start(logits_o.ap()[j], lgs)

    nc.compile()
    return nc


_NC_CACHE = None


def _get_nc():
    global _NC_CACHE
    if _NC_CACHE is None:
        _NC_CACHE = _build_nc()
    return _NC_CACHE


def _host_tables():
    perm = np.concatenate([np.arange(0, N, 2), np.arange(1, N, 2)])
    tq = np.floor(np.arange(N, dtype=np.float64) / 2.0) * 2.0
    freqs = 1.0 / (2.0 ** 16) ** (tq / N) / TWO_PI
    phases = np.arange(T)[None, :] * freqs[:, None]      # (N, T)
    p = (phases % 1.0) * TWO_PI
    cosT = np.cos(p)[perm][: N // 2].astype(BF16)        # (N/2, T); halves identical
    sinT = np.sin(p)[perm][: N // 2].astype(BF16)
    return perm, cosT.reshape(HCH, 128, T), sinT.reshape(HCH, 128, T)


def make_in_maps(idx, embed, encoder, encoder_v, decoder, lm_head):
    perm, cos_t, sin_t = _host_tables()
    idx = np.asarray(idx)
    embed = np.asarray(embed, dtype=np.float32)
    enc = np.asarray(encoder, dtype=np.float32)[:, :, perm].astype(BF16)
    encv = np.asarray(encoder_v, dtype=np.float32)[:, :, perm].astype(BF16)
    dec = np.asarray(decoder, dtype=np.float32).reshape(NH, N, D)[:, perm, :].astype(BF16)
    emb_b = embed.astype(BF16).reshape(VC, 128, D)
    lmh_b = np.asarray(lm_head, dtype=np.float32).astype(BF16).reshape(DC, 128, VOCAB)

    oneh = np.zeros((B, VOCAB, T), dtype=BF16)           # (b, v, t) = onehot^T
    for b in range(B):
        oneh[b, np.asarray(idx[b], dtype=np.int64), np.arange(T)] = 1

    in_maps = []
    for c in range(8):
        b, h = c // 4, c % 4
        in_maps.append({
            "enc_w": np.ascontiguousarray(enc[h].reshape(DC, 128, N)),
            "encv_w": np.ascontiguousarray(encv[h].reshape(DC, 128, N)),
            "dec_w": np.ascontiguousarray(dec[h].reshape(NCH, 128, D)),
            "cos_w": cos_t,
            "sin_w": sin_t,
            "oneh_w": np.ascontiguousarray(oneh[b].reshape(VC, 128, T)),
            "emb_w": emb_b,
            "lmh_w": lmh_b,
        })
    return in_maps


def kernel(idx, embed, encoder, encoder_v, decoder, lm_head):
    nc = _get_nc()
    in_maps = make_in_maps(idx, embed, encoder, encoder_v, decoder, lm_head)
    res = bass_utils.run_bass_kernel_spmd(nc, in_maps, core_ids=list(range(8)))
    out = np.empty((B, T, VOCAB), dtype=np.float32)
    for b in range(B):
        out[b] = res.results[4 * b]["logits_o"].reshape(T, VOCAB)
    return out


# revision 38
# speedup vs baseline: 5315.7606x; 5315.7606x over previous
"""Trainium2 Bass kernel for the BDH dense transformer (B=2, T=512, D=256, NH=4,
N=8192, 4 weight-tied layers, vocab 256).

Sharding: one (batch, head) pair per NeuronCore (2 x 4 = 8 cores). Per layer,
each core computes its head's yMLP partial (T, D); the 4 cores of a batch group
exchange partials with an AllGather and reduce locally, then every core in the
group redundantly applies the residual + layernorm so the activations stay
replicated within the group.

RoPE trick: the score contraction over the neuron dim N is invariant under any
permutation of N applied consistently to (encoder cols, encoder_v cols, decoder
rows, rope freqs). We de-interleave N (even indices first, then odd) so the
interleaved-pair rope becomes rotate-half form: the partner of partition-chunk
j is chunk j+/-32 — a whole-chunk offset instead of an adjacent-partition
shuffle. cos/sin tables are identical for both halves, so only (N/2, T) tables
are streamed.

All matmuls run in bf16 with fp32 PSUM accumulation; layernorms and the
residual stream are fp32.
"""

import math

import numpy as np
import ml_dtypes

import concourse.bass as bass
import concourse.mybir as mybir
import concourse.tile as tile
from concourse import bacc
from concourse import bass_utils
from concourse.masks import make_identity

BF16 = ml_dtypes.bfloat16
F32 = mybir.dt.float32
BF = mybir.dt.bfloat16

# model dims (hardcoded per the problem spec)
B, T, D, NH, VOCAB = 2, 512, 256, 4, 256
N_LAYER = 4
MLP_MULT = 128
N = D * MLP_MULT // NH          # 8192 neurons per head
LN_EPS = 1e-5
TWO_PI = 2.0 * math.pi

NCH = N // 128                   # 64 partition-chunks of the neuron dim
HCH = NCH // 2                   # 32 chunks per rotate-half half
NGRP = 16                        # rope groups of 2 lo-chunks + 2 hi-chunks
TC = T // 128                    # 4 t-chunks
DC = D // 128                    # 2 d-chunks
VC = VOCAB // 128                # 2 vocab-chunks

RG = [[0, 1, 2, 3], [4, 5, 6, 7]]

AF = mybir.ActivationFunctionType
ALU = mybir.AluOpType


def _layer_norm(nc, tmp, eps_tile, src, out_bf, out_f32=None):
    """LN over the free dim (256 wide) of a (128, 256) f32 tile (SBUF or PSUM)."""
    stats = tmp.tile([128, 6], F32, tag="bnst", bufs=2, name="ln_stats")
    nc.vector.bn_stats(stats, src)
    mv = tmp.tile([128, 2], F32, tag="bnmv", bufs=2, name="ln_mv")
    nc.vector.bn_aggr(mv, stats)
    std = tmp.tile([128, 1], F32, tag="std", bufs=2, name="ln_std")
    nc.scalar.activation(std, mv[:, 1:2], AF.Sqrt, bias=eps_tile)
    rstd = tmp.tile([128, 1], F32, tag="rstd", bufs=2, name="ln_rstd")
    nc.vector.reciprocal(rstd, std)
    first_out = out_f32 if out_f32 is not None else out_bf
    nc.vector.tensor_scalar(
        out=first_out, in0=src, scalar1=mv[:, 0:1], scalar2=rstd,
        op0=ALU.subtract, op1=ALU.mult,
    )
    if out_f32 is not None:
        nc.scalar.copy(out_bf, out_f32)


def _build_nc():
    nc = bacc.Bacc("TRN2", target_bir_lowering=False, debug=False, num_devices=8)

    # per-core external inputs
    enc_w = nc.dram_tensor("enc_w", [DC, 128, N], BF, kind="ExternalInput")
    encv_w = nc.dram_tensor("encv_w", [DC, 128, N], BF, kind="ExternalInput")
    dec_w = nc.dram_tensor("dec_w", [NCH, 128, D], BF, kind="ExternalInput")
    cos_w = nc.dram_tensor("cos_w", [HCH, 128, T], BF, kind="ExternalInput")
    sin_w = nc.dram_tensor("sin_w", [HCH, 128, T], BF, kind="ExternalInput")
    oneh_w = nc.dram_tensor("oneh_w", [VC, 128, T], BF, kind="ExternalInput")
    emb_w = nc.dram_tensor("emb_w", [VC, 128, D], BF, kind="ExternalInput")
    lmh_w = nc.dram_tensor("lmh_w", [DC, 128, VOCAB], BF, kind="ExternalInput")
    logits_o = nc.dram_tensor("logits_o", [TC, 128, VOCAB], F32, kind="ExternalOutput")

    with tile.TileContext(nc) as tc:
        with (
            tc.tile_pool(name="wpool", bufs=1) as wp,
            tc.tile_pool(name="xspool", bufs=1) as xsp,
            tc.tile_pool(name="stream", bufs=3) as stream,
            tc.tile_pool(name="rope", bufs=2) as rope,
            tc.tile_pool(name="work", bufs=1) as work,
            tc.tile_pool(name="tmp", bufs=2) as tmp,
            tc.tile_pool(name="psA", bufs=2, space="PSUM") as psA,
            tc.tile_pool(name="psS", bufs=1, space="PSUM") as psS,
            tc.tile_pool(name="psT", bufs=2, space="PSUM") as psT,
            tc.tile_pool(name="dram", bufs=1, space="DRAM") as dram,
        ):
            # ---- resident weights (small tensors first so the embedding can
            # start while the big encoder DMAs stream in) ----
            emb_sb = wp.tile([128, VC, D], BF, name="emb_sb")
            nc.sync.dma_start(emb_sb, emb_w.ap().rearrange("c p d -> p c d"))
            oneh_sb = wp.tile([128, VC, T], BF, name="oneh_sb")
            nc.sync.dma_start(oneh_sb, oneh_w.ap().rearrange("c p t -> p c t"))

            # warm up the collectives path (same payload size as the per-layer
            # AllReduce) so layer 0 doesn't pay the first-call cost; no data
            # deps, overlaps the weight DMAs
            wcc_in = dram.tile([128, TC, D], BF, tag="wccin", name="wcc_in")
            wcc_out = dram.tile([128, TC, D], BF, tag="wccout", name="wcc_out")
            wcc_sb = tmp.tile([128, TC, D], BF, tag="wcc", bufs=1, name="wcc_sb")
            nc.vector.memset(wcc_sb, 0.0)
            nc.sync.dma_start(wcc_in, wcc_sb)

            nc.gpsimd.collective_compute(
                "AllReduce", ALU.add, replica_groups=RG,
                ins=[wcc_in.opt()], outs=[wcc_out.opt()],
            )
            lmh_sb = wp.tile([128, DC, VOCAB], BF, name="lmh_sb")
            nc.sync.dma_start(lmh_sb, lmh_w.ap().rearrange("c p v -> p c v"))
            ident = wp.tile([128, 128], BF, name="ident")
            make_identity(nc, ident)
            eps_tile = wp.tile([128, 1], F32, name="eps_tile")
            nc.vector.memset(eps_tile, LN_EPS)
            # encoder DMAs split into column slices so phase A's first matmuls
            # only wait for the slice they read (subtile deps)
            enc_sb = []
            encv_sb = []
            for dc in range(DC):
                e = wp.tile([128, N], BF, tag=f"enc{dc}", name=f"enc_sb{dc}")
                enc_sb.append(e)
                ev = wp.tile([128, N], BF, tag=f"encv{dc}", name=f"encv_sb{dc}")
                encv_sb.append(ev)
            NSL = 8
            sl = N // NSL
            for s in range(NSL):
                for dc in range(DC):
                    nc.sync.dma_start(
                        enc_sb[dc][:, s * sl:(s + 1) * sl],
                        enc_w.ap()[dc, :, s * sl:(s + 1) * sl],
                    )
            for s in range(NSL):
                for dc in range(DC):
                    nc.sync.dma_start(
                        encv_sb[dc][:, s * sl:(s + 1) * sl],
                        encv_w.ap()[dc, :, s * sl:(s + 1) * sl],
                    )

            # ---- persistent activations ----
            xs = xsp.tile([128, NCH, T], BF, name="xs")          # x_sparse / xy gate
            smask = work.tile([128, TC, T], BF, name="smask")    # masked scores (lhsT)
            x_f32 = work.tile([128, TC, D], F32, name="x_f32")
            x_bf = work.tile([128, TC, D], BF, name="x_bf")
            xT = work.tile([128, DC, T], BF, name="xT")
            ykv_bf = work.tile([128, TC, D], BF, name="ykv_bf")
            ykvT = work.tile([128, DC, T], BF, name="ykvT")
            part_bf = work.tile([128, TC, D], BF, name="part_bf")

            # zero the always-zero lower-left region of the masked score tiles
            for i in range(1, TC):
                nc.vector.memset(smask[:, i, 0:128 * i], 0.0)

            def transpose_to(dst, src_bf):
                # src_bf: (128, TC, D) bf16 (t-part, d-free) -> dst (128, DC, T)
                for j in range(TC):
                    for dc in range(DC):
                        tp = psT.tile([128, 128], BF, tag="tr", name="tp_ps")
                        nc.tensor.transpose(
                            tp, src_bf[:, j, 128 * dc:128 * (dc + 1)], ident
                        )
                        nc.scalar.copy(dst[:, dc, 128 * j:128 * (j + 1)], tp)

            # ---- embedding: x0 = LN(onehot.T @ embed) ----
            for j in range(TC):
                x0 = psT.tile([128, D], F32, tag="tr", name="x0_ps")
                for vc in range(VC):
                    nc.tensor.matmul(
                        x0, lhsT=oneh_sb[:, vc, 128 * j:128 * (j + 1)],
                        rhs=emb_sb[:, vc, :],
                        start=(vc == 0), stop=(vc == VC - 1),
                    )
                _layer_norm(nc, tmp, eps_tile, x0, x_bf[:, j, :], x_f32[:, j, :])
            transpose_to(xT, x_bf)

            # ---- layers ----
            # phase-A chunk order interleaves the rotate-half partners
            # (0,32,1,33,...) so rope group g unblocks after 4g+4 chunks
            a_order = [k for pair in zip(range(HCH), range(HCH, NCH)) for k in pair]

            for layer in range(N_LAYER):
                # phase A: x_latent^T = enc^T @ x^T ; x_sparse = relu
                with nc.named_scope(f"L{layer}_A"):
                    for ki, k in enumerate(a_order):
                        lat = psA.tile([128, T], F32, tag="lat", name="lat_ps")
                        if ki < 4:
                            # split over T so the first half only depends on
                            # the first two t-chunks of xT (overlaps the E
                            # tail of the previous layer); costs extra
                            # LDWEIGHTS so only the leading chunks do it
                            for h in range(2):
                                hs = slice(256 * h, 256 * (h + 1))
                                for dc in range(DC):
                                    nc.tensor.matmul(
                                        lat[:, hs],
                                        lhsT=enc_sb[dc][:, 128 * k:128 * (k + 1)],
                                        rhs=xT[:, dc, hs],
                                        start=(dc == 0), stop=(dc == DC - 1),
                                    )
                        else:
                            for dc in range(DC):
                                nc.tensor.matmul(
                                    lat,
                                    lhsT=enc_sb[dc][:, 128 * k:128 * (k + 1)],
                                    rhs=xT[:, dc, :],
                                    start=(dc == 0), stop=(dc == DC - 1),
                                )
                        nc.scalar.activation(xs[:, k, :], lat, AF.Relu)

                # phase B: rope + scores S = QR^T QR (upper-triangular blocks only)
                spsum = [
                    psS.tile([128, T], F32, tag=f"s{i}", name=f"score_ps{i}")
                    for i in range(TC)
                ]
                GW = 4           # rope group width (chunks per half)
                NG = HCH // GW
                with nc.named_scope(f"L{layer}_B"):
                    for g in range(NG):
                        lo = slice(GW * g, GW * (g + 1))
                        hi = slice(GW * g + HCH, GW * (g + 1) + HCH)
                        cosg = rope.tile([128, GW, T], BF, tag="cosg", name="cosg")
                        nc.sync.dma_start(
                            cosg,
                            cos_w.ap()[GW * g:GW * (g + 1)].rearrange("c p t -> p c t"),
                        )
                        sing = rope.tile([128, GW, T], BF, tag="sing", name="sing")
                        nc.sync.dma_start(
                            sing,
                            sin_w.ap()[GW * g:GW * (g + 1)].rearrange("c p t -> p c t"),
                        )
                        qrlo = rope.tile([128, GW, T], BF, tag="qrlo", name="qrlo")
                        qrhi = rope.tile([128, GW, T], BF, tag="qrhi", name="qrhi")
                        ta = rope.tile([128, GW, T], BF, tag="ta", bufs=1, name="ropeta")
                        tb = rope.tile([128, GW, T], BF, tag="tb", bufs=1, name="ropetb")
                        nc.vector.tensor_mul(ta, xs[:, lo, :], cosg)
                        nc.vector.tensor_mul(tb, xs[:, hi, :], sing)
                        nc.vector.tensor_sub(qrlo, ta, tb)
                        ta2 = rope.tile([128, GW, T], BF, tag="ta", bufs=1, name="ropeta2")
                        tb2 = rope.tile([128, GW, T], BF, tag="tb", bufs=1, name="ropetb2")
                        nc.vector.tensor_mul(ta2, xs[:, hi, :], cosg)
                        nc.vector.tensor_mul(tb2, xs[:, lo, :], sing)
                        nc.vector.tensor_add(qrhi, ta2, tb2)
                        for qr, base in ((qrlo, GW * g), (qrhi, GW * g + HCH)):
                            for kk in range(GW):
                                first = (g == 0) and (qr is qrlo) and (kk == 0)
                                last = (g == NG - 1) and (qr is qrhi) and (kk == GW - 1)
                                for i in range(TC):
                                    nc.tensor.matmul(
                                        spsum[i][:, 128 * i:T],
                                        lhsT=qr[:, kk, 128 * i:128 * (i + 1)],
                                        rhs=qr[:, kk, 128 * i:T],
                                        start=first, stop=last,
                                    )

                # phase C: mask scores, attention out, LN, transpose
                with nc.named_scope(f"L{layer}_C"):
                    for i in range(TC):
                        if i % 2 == 0:
                            nc.vector.tensor_copy(
                                out=smask[:, i, 128 * i:T],
                                in_=spsum[i][:, 128 * i:T],
                            )
                        else:
                            nc.scalar.copy(
                                smask[:, i, 128 * i:T], spsum[i][:, 128 * i:T]
                            )
                        diag = smask[:, i, 128 * i:128 * (i + 1)]
                        nc.gpsimd.affine_select(
                            out=diag, in_=diag, pattern=[[1, 128]], base=0,
                            channel_multiplier=-1, compare_op=ALU.is_gt, fill=0.0,
                        )
                    for j in range(TC):
                        att = psT.tile([128, D], F32, tag="tr", name="att_ps")
                        for i in range(j + 1):
                            nc.tensor.matmul(
                                att, lhsT=smask[:, i, 128 * j:128 * (j + 1)],
                                rhs=x_bf[:, i, :],
                                start=(i == 0), stop=(i == j),
                            )
                        _layer_norm(nc, tmp, eps_tile, att, ykv_bf[:, j, :])
                    transpose_to(ykvT, ykv_bf)

                # phase D: y_latent, relu, gate, yMLP partial
                mlp = [
                    psS.tile([128, D], F32, tag=f"s{j}", name=f"mlp_ps{j}")
                    for j in range(TC)
                ]
                with nc.named_scope(f"L{layer}_D"):
                    dec_pair = None
                    for k in range(NCH):
                        if k % 2 == 0:
                            dec_pair = stream.tile(
                                [128, 2, D], BF, tag="dec", bufs=4, name="dec_t"
                            )
                            nc.sync.dma_start(
                                dec_pair,
                                dec_w.ap()[k:k + 2].rearrange("c p d -> p c d"),
                            )
                        dec_t = dec_pair[:, k % 2, :]
                        ylat = psA.tile([128, T], F32, tag="lat", name="ylat_ps")
                        for dc in range(DC):
                            nc.tensor.matmul(
                                ylat,
                                lhsT=encv_sb[dc][:, 128 * k:128 * (k + 1)],
                                rhs=ykvT[:, dc, :],
                                start=(dc == 0), stop=(dc == DC - 1),
                            )
                        ys = stream.tile([128, T], BF, tag="ys", name="ys_t")
                        nc.scalar.activation(ys, ylat, AF.Relu)
                        nc.vector.tensor_mul(xs[:, k, :], xs[:, k, :], ys)
                        for j in range(TC):
                            nc.tensor.matmul(
                                mlp[j], lhsT=xs[:, k, 128 * j:128 * (j + 1)],
                                rhs=dec_t,
                                start=(k == 0), stop=(k == NCH - 1),
                            )

                # phase E: exchange partials, reduce, residual + LN, transpose
                with nc.named_scope(f"L{layer}_E"):
                    for j in range(TC):
                        if j % 2 == 0:
                            nc.scalar.copy(part_bf[:, j, :], mlp[j])
                        else:
                            nc.vector.tensor_copy(out=part_bf[:, j, :], in_=mlp[j])
                    cc_in = dram.tile([128, TC, D], BF, tag="ccin", name="cc_in")
                    cc_out = dram.tile([128, TC, D], BF, tag="ccout", name="cc_out")
                    nc.sync.dma_start(cc_in, part_bf)
                    nc.gpsimd.collective_compute(
                        "AllReduce", ALU.add, replica_groups=RG,
                        ins=[cc_in.opt()], outs=[cc_out.opt()],
                    )
                    ag = tmp.tile([128, TC, D], BF, tag="ag", bufs=1, name="ag")
                    nc.sync.dma_start(ag[:, 0:2, :], cc_out[:, 0:2, :])
                    nc.sync.dma_start(ag[:, 2:4, :], cc_out[:, 2:4, :])
                    for j in range(TC):
                        xsum = tmp.tile([128, D], F32, tag="xsum", name="xsum")
                        nc.vector.tensor_add(xsum, ag[:, j, :], x_f32[:, j, :])
                        _layer_norm(
                            nc, tmp, eps_tile, xsum,
                            x_bf[:, j, :], x_f32[:, j, :],
                        )
                        for dc in range(DC):
                            tp = psT.tile([128, 128], BF, tag="tr", name="tp_ps")
                            nc.tensor.transpose(
                                tp, x_bf[:, j, 128 * dc:128 * (dc + 1)], ident
                            )
                            nc.scalar.copy(
                                xT[:, dc, 128 * j:128 * (j + 1)], tp
                            )
                        if layer == N_LAYER - 1:
                            # lm head per chunk, overlapping the rest of the
                            # final exchange tail
                            lg = psT.tile([128, VOCAB], F32, tag="tr", name="lg_ps")
                            for dc in range(DC):
                                nc.tensor.matmul(
                                    lg, lhsT=xT[:, dc, 128 * j:128 * (j + 1)],
                                    rhs=lmh_sb[:, dc, :],
                                    start=(dc == 0), stop=(dc == DC - 1),
                                )
                            lgs = tmp.tile([128, VOCAB], F32, tag="lgs", name="lg_sb")
                            nc.scalar.copy(lgs, lg)
                            nc.sync.dma_start(logits_o.ap()[j], lgs)

    nc.compile()
    return nc


_NC_CACHE = None


def _get_nc():
    global _NC_CACHE
    if _NC_CACHE is None:
        _NC_CACHE = _build_nc()
    return _NC_CACHE


def _host_tables():
    perm = np.concatenate([np.arange(0, N, 2), np.arange(1, N, 2)])
    tq = np.floor(np.arange(N, dtype=np.float64) / 2.0) * 2.0
    freqs = 1.0 / (2.0 ** 16) ** (tq / N) / TWO_PI
    phases = np.arange(T)[None, :] * freqs[:, None]      # (N, T)
    p = (phases % 1.0) * TWO_PI
    cosT = np.cos(p)[perm][: N // 2].astype(BF16)        # (N/2, T); halves identical
    sinT = np.sin(p)[perm][: N // 2].astype(BF16)
    return perm, cosT.reshape(HCH, 128, T), sinT.reshape(HCH, 128, T)


def make_in_maps(idx, embed, encoder, encoder_v, decoder, lm_head):
    perm, cos_t, sin_t = _host_tables()
    idx = np.asarray(idx)
    embed = np.asarray(embed, dtype=np.float32)
    enc = np.asarray(encoder, dtype=np.float32)[:, :, perm].astype(BF16)
    encv = np.asarray(encoder_v, dtype=np.float32)[:, :, perm].astype(BF16)
    dec = np.asarray(decoder, dtype=np.float32).reshape(NH, N, D)[:, perm, :].astype(BF16)
    emb_b = embed.astype(BF16).reshape(VC, 128, D)
    lmh_b = np.asarray(lm_head, dtype=np.float32).astype(BF16).reshape(DC, 128, VOCAB)

    oneh = np.zeros((B, VOCAB, T), dtype=BF16)           # (b, v, t) = onehot^T
    for b in range(B):
        oneh[b, np.asarray(idx[b], dtype=np.int64), np.arange(T)] = 1

    in_maps = []
    for c in range(8):
        b, h = c // 4, c % 4
        in_maps.append({
            "enc_w": np.ascontiguousarray(enc[h].reshape(DC, 128, N)),
            "encv_w": np.ascontiguousarray(encv[h].reshape(DC, 128, N)),
            "dec_w": np.ascontiguousarray(dec[h].reshape(NCH, 128, D)),
            "cos_w": cos_t,
            "sin_w": sin_t,
            "oneh_w": np.ascontiguousarray(oneh[b].reshape(VC, 128, T)),
            "emb_w": emb_b,
            "lmh_w": lmh_b,
        })
    return in_maps


def kernel(idx, embed, encoder, encoder_v, decoder, lm_head):
    nc = _get_nc()
    in_maps = make_in_maps(idx, embed, encoder, encoder_v, decoder, lm_head)
    res = bass_utils.run_bass_kernel_spmd(nc, in_maps, core_ids=list(range(8)))
    out = np.empty((B, T, VOCAB), dtype=np.float32)
    for b in range(B):
        out[b] = res.results[4 * b]["logits_o"].reshape(T, VOCAB)
    return out
